# revision 29
# baseline (speedup 1.0000x reference)
"""Trainium2 Bass kernel for nn_EnergyMapping (per-edge MLP -> energy sum).

Math (per molecule b):
    pre  = edge_embedding @ W1 + b1            # (E, H) with E = At*Nbr edges
    g    = softplus(pre)                        # shifted_softplus = g - log(2)
    y_e  = (g_e - log2) @ W2 + b2               # per-edge scalar
    E_b  = sum_e y_e
         = sum_h W2[h] * S[b,h] - E*log2*sum(W2) + E*b2,   S[b,h] = sum_e g[b,e,h]

Strategy: data-parallel over the batch dim (16 molecules / 8 cores = 2 each).
Each core receives its shard pre-transposed to [F=128, E=32768] so the
contraction dim F sits on SBUF partitions with perfectly contiguous DMA.
On-device per core (DMA-bound: 16 MiB @ ~350 GB/s ~= 48 us floor):
  - W1 [128, 64] is the stationary operand (natural layout = lhsT).
  - Stream X^T in [128, 4096] chunks (4x 512 KiB sub-DMAs so matmuls start
    on the first quarter while the rest lands).
  - Matmul pairs of 512-edge groups into PSUM [128, 1024] tiles (2 banks)
    via column tiling: group A -> partitions 0:64, group B -> 64:128; the
    two M=64 matmuls run concurrently in the PE array, doubling fp32
    TensorE throughput.
  - softplus = ln(1 + exp(x)) in two wide ScalarE passes (both functions in
    the single natural_log_exp_and_others ACT table set -- see _EnergyBacc);
    the Ln pass covers a whole 4096-edge chunk and emits the per-partition
    row sum for free via accum_out into one accumulator slot per chunk.
  - Only the [128, 8] slot accumulator leaves the device; the final tiny
    dot with W2 and the b2/log2 corrections happen on host (fp64).
Measured steady-state ~52 us/exec per core vs ~48.5 us pure-DMA floor.
"""

import numpy as np

import concourse.bass as bass
import concourse.mybir as mybir
import concourse.tile as tile
from concourse import bacc
from concourse.bass_utils import run_bass_kernel_spmd

# Problem shapes (fixed by the task; kernel.py must be self-contained).
B, At, Nbr, F = 16, 256, 64, 128
H = F // 2                       # 64
N_CORES = 8
B_PER_CORE = B // N_CORES        # 2 molecules per core
EDGES_PER_MOL = At * Nbr         # 16384
E_PER_CORE = B_PER_CORE * EDGES_PER_MOL  # 32768

GROUP = 512                      # moving free dim per matmul (fp32 max, 1 PSUM bank)
PSUM_WIDE = 2 * GROUP            # psum tile free size (2 banks; holds 2048 edges)
LN_WIDE = 2 * PSUM_WIDE          # Ln pass width in columns
CHUNK = 4096                     # edges per DMA chunk (2 MiB transfers)
N_CHUNKS = E_PER_CORE // CHUNK   # 8
# One Ln (+accum slot) covers a whole chunk: 2*LN_WIDE = CHUNK edges
# (each column position holds 2 edges via the partition halves).
N_SLOTS = N_CHUNKS               # 8 accumulator slots, slot c == chunk c
SLOTS_PER_MOL = EDGES_PER_MOL // CHUNK  # 4

LOG2 = float(np.log(2.0))

# "native": single ScalarE Softplus LUT pass — NOT supported by this
#   toolchain's act_info.json (no softplus func set) -> walrus lowering fails.
# "explog": two passes, exp then ln(1+t); both funcs live in the
#   natural_log_exp_and_others ACT table set, so no table switching.
SOFTPLUS_MODE = "explog"

_NC_CACHE = {}

# Both halves of softplus = ln(1 + exp(x)) live in this ACT table set. The
# default table-load pass picks the first set containing each function
# (exp -> exp_and_others, ln -> natural_log), which inserts a ~1.3us
# LoadActFuncSet before nearly every activation (~80us/core!). Restricting
# the candidate tables to the combined set keeps one load for the whole
# kernel. Other sets are blanked (not removed) so act_func_set_id indices
# into act_info.json stay valid.
_ACT_SET_BOTH = "natural_log_exp_and_others"


class _EnergyBacc(bacc.Bacc):
    def insert_act_table_loads(self):
        import bass_rust as _bass_rust
        from concourse.hw_specs import get_activation_tables

        has_activation = any(
            isinstance(i, mybir.InstActivation)
            for b in self.main_func.blocks
            for i in b.instructions
        )
        if not has_activation:
            return
        all_tables = get_activation_tables(self.m.arch)
        if _ACT_SET_BOTH in all_tables:
            tables = [
                (name, funcs if name == _ACT_SET_BOTH else set())
                for name, funcs in all_tables.items()
            ]
        else:  # unexpected toolchain: fall back to default behaviour
            tables = list(all_tables.items())
        _bass_rust.insert_act_table_loads(self, tables)


def _build_nc(softplus_mode: str, reps: int = 1, loop: int = 0, parts: str = "full",
              xbufs: int = 3, psbufs: int = 3, gbufs: int = 3,
              dma_split: int = 4) -> bass.Bass:
    """Build the per-core Bass program. reps>1 repeats the whole kernel body
    unrolled; loop>0 wraps the body in a For_i hardware loop. Both are used
    only for slope-based HW timing; the output is just overwritten."""
    from contextlib import ExitStack

    nc = _EnergyBacc("TRN2", target_bir_lowering=False, debug=False)
    f32 = mybir.dt.float32
    xt = nc.dram_tensor("xt", [F, E_PER_CORE], f32, kind="ExternalInput")
    w1 = nc.dram_tensor("w1", [F, H], f32, kind="ExternalInput")
    b1c = nc.dram_tensor("b1c", [128, 1], f32, kind="ExternalInput")
    acc = nc.dram_tensor("acc", [128, N_SLOTS], f32, kind="ExternalOutput")

    with tile.TileContext(nc) as tc:
        with ExitStack() as ctx:
            consts = ctx.enter_context(tc.tile_pool(name="consts", bufs=1))
            xpool = ctx.enter_context(tc.tile_pool(name="xpool", bufs=xbufs))
            psum = ctx.enter_context(tc.tile_pool(name="psum", bufs=psbufs, space="PSUM"))
            gpool = ctx.enter_context(tc.tile_pool(name="gpool", bufs=gbufs))
            opool = ctx.enter_context(tc.tile_pool(name="opool", bufs=1))

            w1_sb = consts.tile([F, H], f32)
            nc.sync.dma_start(w1_sb[:], w1[:, :])
            b1_sb = consts.tile([128, 1], f32)
            nc.sync.dma_start(b1_sb[:], b1c[:, :])

            acc_sb = opool.tile([128, N_SLOTS], f32)

            if loop:
                ctx.enter_context(tc.For_i(0, loop, 1))

            for _rep in range(reps):
                # Zero-init: makes overwrite-vs-accumulate accum_out semantics
                # equivalent (each slot is written by exactly one instruction).
                nc.vector.memset(acc_sb[:], 0.0)

                for c in range(N_CHUNKS):
                    xtile = xpool.tile([F, CHUNK], f32, tag="xtile")
                    part = CHUNK // dma_split
                    for s in range(dma_split):
                        nc.sync.dma_start(
                            xtile[:, s * part : (s + 1) * part],
                            xt[:, c * CHUNK + s * part : c * CHUNK + (s + 1) * part],
                        )
                    if parts == "dma":
                        continue
                    # t accumulates exp() for the whole chunk; one wide Ln
                    # (+free row-sum accum) finishes softplus per chunk.
                    t = gpool.tile([128, LN_WIDE], f32, tag="t")
                    # each psum tile holds 2*PSUM_WIDE edges (2 per column
                    # position, via the partition halves)
                    for k in range(CHUNK // (2 * PSUM_WIDE)):
                        e0 = k * 2 * PSUM_WIDE
                        ps = psum.tile([128, PSUM_WIDE], f32, tag="ps")
                        # Column-tiled pairs: M=64 matmuls land on disjoint
                        # PSUM partition halves and run concurrently in the
                        # PE array; each [64, 512] output fits one bank.
                        for q in range(PSUM_WIDE // GROUP):
                            g0 = e0 + 2 * q * GROUP
                            nc.tensor.matmul(
                                ps[0:64, q * GROUP : (q + 1) * GROUP],
                                w1_sb[:], xtile[:, g0 : g0 + GROUP],
                                start=True, stop=True,
                            )
                            nc.tensor.matmul(
                                ps[64:128, q * GROUP : (q + 1) * GROUP],
                                w1_sb[:], xtile[:, g0 + GROUP : g0 + 2 * GROUP],
                                start=True, stop=True,
                            )
                        if parts == "dma+mm":
                            continue  # no psum consumer; PE self-serializes
                        nc.scalar.activation(
                            t[:, k * PSUM_WIDE : (k + 1) * PSUM_WIDE], ps[:],
                            mybir.ActivationFunctionType.Exp,
                            bias=b1_sb[:], scale=1.0,
                        )
                    if parts == "dma+mm":
                        continue
                    g = gpool.tile([128, LN_WIDE], f32, tag="g")
                    nc.scalar.activation(
                        g[:], t[:],
                        mybir.ActivationFunctionType.Ln,
                        bias=1.0, scale=1.0,
                        accum_out=acc_sb[:, c : c + 1],
                    )

                nc.sync.dma_start(acc[:, :], acc_sb[:])
    nc.compile()
    return nc


def _get_nc() -> bass.Bass:
    if SOFTPLUS_MODE not in _NC_CACHE:
        _NC_CACHE[SOFTPLUS_MODE] = _build_nc(SOFTPLUS_MODE)
    return _NC_CACHE[SOFTPLUS_MODE]


def _make_in_maps(edge_embedding, W1, b1):
    X = np.ascontiguousarray(edge_embedding, dtype=np.float32).reshape(B, EDGES_PER_MOL, F)
    w1 = np.ascontiguousarray(W1, dtype=np.float32)
    b1c = np.concatenate([np.asarray(b1, np.float32)] * 2).reshape(128, 1)
    b1c = np.ascontiguousarray(b1c)
    in_maps = []
    for c in range(N_CORES):
        xc = X[c * B_PER_CORE : (c + 1) * B_PER_CORE].reshape(E_PER_CORE, F)
        xtc = np.ascontiguousarray(xc.T)  # [F, E] shard, F on partitions
        in_maps.append({"xt": xtc, "w1": w1, "b1c": b1c})
    return in_maps


def _finalize(results, W1, b1, W2, b2):
    W2v = np.asarray(W2, np.float64).reshape(H)
    b2v = float(np.asarray(b2).reshape(()))
    out = np.empty((B, 1), np.float32)
    corr = -EDGES_PER_MOL * LOG2 * float(W2v.sum()) + EDGES_PER_MOL * b2v
    for c in range(N_CORES):
        acc = np.asarray(results[c]["acc"], np.float64)  # [128, N_SLOTS]
        S = acc[0:64, :] + acc[64:128, :]  # per-h, per-slot softplus sums
        for i in range(B_PER_CORE):
            b = c * B_PER_CORE + i
            Sg = S[:, i * SLOTS_PER_MOL : (i + 1) * SLOTS_PER_MOL].sum(axis=1)
            out[b, 0] = np.float32(Sg @ W2v + corr)
    return out


def kernel_with_results(edge_embedding, W1, b1, W2, b2, trace=False, **run_kwargs):
    nc = _get_nc()
    in_maps = _make_in_maps(edge_embedding, W1, b1)
    core_ids = list(range(N_CORES))
    try:
        br = run_bass_kernel_spmd(nc, in_maps, core_ids, trace=trace, **run_kwargs)
    except ModuleNotFoundError:
        # Slim axon clients lack the NTFF profile hook (antenv.axon_hooks);
        # retry without tracing rather than failing the whole kernel.
        import os
        os.environ["BASS_NEVER_TRACE"] = "1"
        br = run_bass_kernel_spmd(nc, in_maps, core_ids, trace=False, **run_kwargs)
    out = _finalize(br.results, W1, b1, W2, b2)
    return out, br


def kernel(edge_embedding, W1, b1, W2, b2):
    out, _ = kernel_with_results(edge_embedding, W1, b1, W2, b2)
    return out


# revision 36
# speedup vs baseline: 1.0108x; 1.0108x over previous
"""Trainium2 Bass kernel for nn_EnergyMapping (per-edge MLP -> energy sum).

Math (per molecule b):
    pre  = edge_embedding @ W1 + b1            # (E, H) with E = At*Nbr edges
    g    = softplus(pre)                        # shifted_softplus = g - log(2)
    y_e  = (g_e - log2) @ W2 + b2               # per-edge scalar
    E_b  = sum_e y_e
         = sum_h W2[h] * S[b,h] - E*log2*sum(W2) + E*b2,   S[b,h] = sum_e g[b,e,h]

Strategy: data-parallel over the batch dim (16 molecules / 8 cores = 2 each).
Each core receives its shard pre-transposed to [F=128, E=32768] so the
contraction dim F sits on SBUF partitions with perfectly contiguous DMA.
On-device per core (DMA-bound: 16 MiB @ ~350 GB/s ~= 48 us floor):
  - W1 [128, 64] is the stationary operand (natural layout = lhsT).
  - Stream X^T in [128, 4096] chunks (4x 512 KiB sub-DMAs so matmuls start
    on the first quarter while the rest lands).
  - Matmul pairs of 512-edge groups into PSUM [128, 1024] tiles (2 banks)
    via column tiling: group A -> partitions 0:64, group B -> 64:128; the
    two M=64 matmuls run concurrently in the PE array, doubling fp32
    TensorE throughput.
  - softplus = ln(1 + exp(x)) in two wide ScalarE passes (both functions in
    the single natural_log_exp_and_others ACT table set -- see _EnergyBacc);
    the Ln pass covers a whole 4096-edge chunk and emits the per-partition
    row sum for free via accum_out into one accumulator slot per chunk.
  - Only the [128, n_slots] accumulator leaves the device; the final tiny
    dot with W2 and the b2/log2 corrections happen on host (fp64).
  - The last chunk is split in two (TAIL_SPLIT) to halve the serial
    matmul->Exp->Ln tail after the final DMA lands.
Measured steady-state ~52-55 us/exec per core vs ~46-48 us pure-DMA floor
(16 MiB @ ~358 GB/s HBM-per-core limit); session-to-session drift on the
shared terminal is +/-4 us.
"""

import numpy as np

import concourse.bass as bass
import concourse.mybir as mybir
import concourse.tile as tile
from concourse import bacc
from concourse.bass_utils import run_bass_kernel_spmd

# Problem shapes (fixed by the task; kernel.py must be self-contained).
B, At, Nbr, F = 16, 256, 64, 128
H = F // 2                       # 64
N_CORES = 8
B_PER_CORE = B // N_CORES        # 2 molecules per core
EDGES_PER_MOL = At * Nbr         # 16384
E_PER_CORE = B_PER_CORE * EDGES_PER_MOL  # 32768

GROUP = 512                      # moving free dim per matmul (fp32 max, 1 PSUM bank)
PSUM_WIDE = 2 * GROUP            # psum tile free size (2 banks; holds 2048 edges)
LN_WIDE = 2 * PSUM_WIDE          # Ln pass width in columns
CHUNK = 4096                     # edges per DMA chunk (2 MiB transfers)
N_CHUNKS = E_PER_CORE // CHUNK   # 8
# One Ln (+accum slot) covers a whole chunk: 2*LN_WIDE = CHUNK edges
# (each column position holds 2 edges via the partition halves).
N_SLOTS = N_CHUNKS               # 8 accumulator slots, slot c == chunk c
SLOTS_PER_MOL = EDGES_PER_MOL // CHUNK  # 4

LOG2 = float(np.log(2.0))

# "native": single ScalarE Softplus LUT pass — NOT supported by this
#   toolchain's act_info.json (no softplus func set) -> walrus lowering fails.
# "explog": two passes, exp then ln(1+t); both funcs live in the
#   natural_log_exp_and_others ACT table set, so no table switching.
SOFTPLUS_MODE = "explog"

_NC_CACHE = {}

# Both halves of softplus = ln(1 + exp(x)) live in this ACT table set. The
# default table-load pass picks the first set containing each function
# (exp -> exp_and_others, ln -> natural_log), which inserts a ~1.3us
# LoadActFuncSet before nearly every activation (~80us/core!). Restricting
# the candidate tables to the combined set keeps one load for the whole
# kernel. Other sets are blanked (not removed) so act_func_set_id indices
# into act_info.json stay valid.
_ACT_SET_BOTH = "natural_log_exp_and_others"


class _EnergyBacc(bacc.Bacc):
    def insert_act_table_loads(self):
        import bass_rust as _bass_rust
        from concourse.hw_specs import get_activation_tables

        has_activation = any(
            isinstance(i, mybir.InstActivation)
            for b in self.main_func.blocks
            for i in b.instructions
        )
        if not has_activation:
            return
        all_tables = get_activation_tables(self.m.arch)
        if _ACT_SET_BOTH in all_tables:
            tables = [
                (name, funcs if name == _ACT_SET_BOTH else set())
                for name, funcs in all_tables.items()
            ]
        else:  # unexpected toolchain: fall back to default behaviour
            tables = list(all_tables.items())
        _bass_rust.insert_act_table_loads(self, tables)


def _chunk_plan(tail_split: bool):
    """Edge counts per chunk. tail_split shortens the serial tail by ending
    with two half chunks. Chunks never straddle a molecule boundary."""
    if tail_split:
        return [CHUNK] * (N_CHUNKS - 1) + [CHUNK // 2, CHUNK // 2]
    return [CHUNK] * N_CHUNKS


def _build_nc(softplus_mode: str, reps: int = 1, loop: int = 0, parts: str = "full",
              xbufs: int = 3, psbufs: int = 3, gbufs: int = 3,
              dma_split: int = 4, tail_split: bool = False,
              dual_dge: bool = False, staggered: bool = False) -> bass.Bass:
    """Build the per-core Bass program. reps>1 repeats the whole kernel body
    unrolled; loop>0 wraps the body in a For_i hardware loop. Both are used
    only for slope-based HW timing; the output is just overwritten."""
    from contextlib import ExitStack

    nc = _EnergyBacc("TRN2", target_bir_lowering=False, debug=False)
    f32 = mybir.dt.float32
    plan = _chunk_plan(tail_split)
    n_slots = len(plan)
    xt = nc.dram_tensor("xt", [F, E_PER_CORE], f32, kind="ExternalInput")
    w1 = nc.dram_tensor("w1", [F, H], f32, kind="ExternalInput")
    b1c = nc.dram_tensor("b1c", [128, 1], f32, kind="ExternalInput")
    acc = nc.dram_tensor("acc", [128, n_slots], f32, kind="ExternalOutput")

    with tile.TileContext(nc) as tc:
        with ExitStack() as ctx:
            consts = ctx.enter_context(tc.tile_pool(name="consts", bufs=1))
            xpool = ctx.enter_context(tc.tile_pool(name="xpool", bufs=xbufs))
            psum = ctx.enter_context(tc.tile_pool(name="psum", bufs=psbufs, space="PSUM"))
            gpool = ctx.enter_context(tc.tile_pool(name="gpool", bufs=gbufs))
            opool = ctx.enter_context(tc.tile_pool(name="opool", bufs=1))

            w1_sb = consts.tile([F, H], f32)
            nc.sync.dma_start(w1_sb[:], w1[:, :])
            b1_sb = consts.tile([128, 1], f32)
            nc.sync.dma_start(b1_sb[:], b1c[:, :])

            acc_sb = opool.tile([128, n_slots], f32)

            if loop:
                ctx.enter_context(tc.For_i(0, loop, 1, staggered_reset=staggered))

            dma_engines = [nc.sync, nc.scalar] if dual_dge else [nc.sync]
            dma_i = 0
            for _rep in range(reps):
                # Zero-init: makes overwrite-vs-accumulate accum_out semantics
                # equivalent (each slot is written by exactly one instruction).
                nc.vector.memset(acc_sb[:], 0.0)

                e_base = 0
                for c, csize in enumerate(plan):
                    xtile = xpool.tile([F, CHUNK], f32, tag="xtile")
                    nsplit = max(1, min(dma_split, csize // 1024))
                    part = csize // nsplit
                    for s in range(nsplit):
                        eng = dma_engines[dma_i % len(dma_engines)]
                        dma_i += 1
                        eng.dma_start(
                            xtile[:, s * part : (s + 1) * part],
                            xt[:, e_base + s * part : e_base + (s + 1) * part],
                        )
                    e_base += csize
                    if parts == "dma":
                        continue
                    # t accumulates exp() for the whole chunk; one wide Ln
                    # (+free row-sum accum) finishes softplus per chunk.
                    cwide = csize // 2  # columns; 2 edges per column position
                    t = gpool.tile([128, LN_WIDE], f32, tag="t")
                    # each psum tile holds 2*PSUM_WIDE edges (2 per column
                    # position, via the partition halves)
                    for k in range(csize // (2 * PSUM_WIDE)):
                        e0 = k * 2 * PSUM_WIDE
                        ps = psum.tile([128, PSUM_WIDE], f32, tag="ps")
                        # Column-tiled pairs: M=64 matmuls land on disjoint
                        # PSUM partition halves and run concurrently in the
                        # PE array; each [64, 512] output fits one bank.
                        for q in range(PSUM_WIDE // GROUP):
                            g0 = e0 + 2 * q * GROUP
                            nc.tensor.matmul(
                                ps[0:64, q * GROUP : (q + 1) * GROUP],
                                w1_sb[:], xtile[:, g0 : g0 + GROUP],
                                start=True, stop=True,
                            )
                            nc.tensor.matmul(
                                ps[64:128, q * GROUP : (q + 1) * GROUP],
                                w1_sb[:], xtile[:, g0 + GROUP : g0 + 2 * GROUP],
                                start=True, stop=True,
                            )
                        if parts == "dma+mm":
                            continue  # no psum consumer; PE self-serializes
                        nc.scalar.activation(
                            t[:, k * PSUM_WIDE : (k + 1) * PSUM_WIDE], ps[:],
                            mybir.ActivationFunctionType.Exp,
                            bias=b1_sb[:], scale=1.0,
                        )
                    if parts == "dma+mm":
                        continue
                    g = gpool.tile([128, LN_WIDE], f32, tag="g")
                    nc.scalar.activation(
                        g[:, :cwide], t[:, :cwide],
                        mybir.ActivationFunctionType.Ln,
                        bias=1.0, scale=1.0,
                        accum_out=acc_sb[:, c : c + 1],
                    )

                nc.sync.dma_start(acc[:, :], acc_sb[:])
    nc.compile()
    return nc


# kernel() uses the tail-split chunk plan: the last 4 MiB chunk becomes two
# 2 MiB chunks, halving the serial matmul->Exp->Ln tail after the final DMA
# (~2-3us off the one-shot execution; steady-state throughput unchanged).
TAIL_SPLIT = True


def _slot_mols(plan):
    """Molecule index owning each accumulator slot (chunks never straddle)."""
    mols, e = [], 0
    for sz in plan:
        mols.append(e // EDGES_PER_MOL)
        e += sz
    return mols


def _get_nc() -> bass.Bass:
    key = (SOFTPLUS_MODE, TAIL_SPLIT)
    if key not in _NC_CACHE:
        _NC_CACHE[key] = _build_nc(SOFTPLUS_MODE, tail_split=TAIL_SPLIT)
    return _NC_CACHE[key]


def _make_in_maps(edge_embedding, W1, b1):
    X = np.ascontiguousarray(edge_embedding, dtype=np.float32).reshape(B, EDGES_PER_MOL, F)
    w1 = np.ascontiguousarray(W1, dtype=np.float32)
    b1c = np.concatenate([np.asarray(b1, np.float32)] * 2).reshape(128, 1)
    b1c = np.ascontiguousarray(b1c)
    in_maps = []
    for c in range(N_CORES):
        xc = X[c * B_PER_CORE : (c + 1) * B_PER_CORE].reshape(E_PER_CORE, F)
        xtc = np.ascontiguousarray(xc.T)  # [F, E] shard, F on partitions
        in_maps.append({"xt": xtc, "w1": w1, "b1c": b1c})
    return in_maps


def _finalize(results, W1, b1, W2, b2):
    W2v = np.asarray(W2, np.float64).reshape(H)
    b2v = float(np.asarray(b2).reshape(()))
    out = np.empty((B, 1), np.float32)
    corr = -EDGES_PER_MOL * LOG2 * float(W2v.sum()) + EDGES_PER_MOL * b2v
    mols = np.array(_slot_mols(_chunk_plan(TAIL_SPLIT)))
    for c in range(N_CORES):
        acc = np.asarray(results[c]["acc"], np.float64)  # [128, n_slots]
        S = acc[0:64, :] + acc[64:128, :]  # per-h, per-slot softplus sums
        for i in range(B_PER_CORE):
            b = c * B_PER_CORE + i
            Sg = S[:, mols == i].sum(axis=1)
            out[b, 0] = np.float32(Sg @ W2v + corr)
    return out


def kernel_with_results(edge_embedding, W1, b1, W2, b2, trace=False, **run_kwargs):
    nc = _get_nc()
    in_maps = _make_in_maps(edge_embedding, W1, b1)
    core_ids = list(range(N_CORES))
    try:
        br = run_bass_kernel_spmd(nc, in_maps, core_ids, trace=trace, **run_kwargs)
    except ModuleNotFoundError:
        # Slim axon clients lack the NTFF profile hook (antenv.axon_hooks);
        # retry without tracing rather than failing the whole kernel.
        import os
        os.environ["BASS_NEVER_TRACE"] = "1"
        br = run_bass_kernel_spmd(nc, in_maps, core_ids, trace=False, **run_kwargs)
    out = _finalize(br.results, W1, b1, W2, b2)
    return out, br


def kernel(edge_embedding, W1, b1, W2, b2):
    out, _ = kernel_with_results(edge_embedding, W1, b1, W2, b2)
    return out


# revision 40
# speedup vs baseline: 1.0562x; 1.0449x over previous
"""Trainium2 Bass kernel for nn_EnergyMapping (per-edge MLP -> energy sum).

Math (per molecule b):
    pre  = edge_embedding @ W1 + b1            # (E, H) with E = At*Nbr edges
    g    = softplus(pre)                        # shifted_softplus = g - log(2)
    y_e  = (g_e - log2) @ W2 + b2               # per-edge scalar
    E_b  = sum_e y_e
         = sum_h W2[h] * S[b,h] - E*log2*sum(W2) + E*b2,   S[b,h] = sum_e g[b,e,h]

Strategy: data-parallel over the batch dim (16 molecules / 8 cores = 2 each).
Each core receives its shard pre-transposed to [F=128, E=32768] so the
contraction dim F sits on SBUF partitions with perfectly contiguous DMA.
On-device per core (DMA-bound: 16 MiB @ ~350 GB/s ~= 48 us floor):
  - W1 [128, 64] is the stationary operand (natural layout = lhsT).
  - Stream X^T in [128, 4096] chunks (4x 512 KiB sub-DMAs so matmuls start
    on the first quarter while the rest lands).
  - Matmul pairs of 512-edge groups into PSUM [128, 1024] tiles (2 banks)
    via column tiling: group A -> partitions 0:64, group B -> 64:128; the
    two M=64 matmuls run concurrently in the PE array, doubling fp32
    TensorE throughput.
  - softplus = ln(1 + exp(x)) in two wide ScalarE passes (both functions in
    the single natural_log_exp_and_others ACT table set -- see _EnergyBacc);
    the Ln pass covers a whole 4096-edge chunk and emits the per-partition
    row sum for free via accum_out into one accumulator slot per chunk.
  - Only the [128, n_slots] accumulator leaves the device; the final tiny
    dot with W2 and the b2/log2 corrections happen on host (fp64).
  - The last chunk is split in two (TAIL_SPLIT) to halve the serial
    matmul->Exp->Ln tail after the final DMA lands.
Measured steady-state ~52-55 us/exec per core vs ~46-48 us pure-DMA floor
(16 MiB @ ~358 GB/s HBM-per-core limit); session-to-session drift on the
shared terminal is +/-4 us.
"""

import numpy as np

import concourse.bass as bass
import concourse.mybir as mybir
import concourse.tile as tile
from concourse import bacc
from concourse.bass_utils import run_bass_kernel_spmd

# Problem shapes (fixed by the task; kernel.py must be self-contained).
B, At, Nbr, F = 16, 256, 64, 128
H = F // 2                       # 64
N_CORES = 8
B_PER_CORE = B // N_CORES        # 2 molecules per core
EDGES_PER_MOL = At * Nbr         # 16384
E_PER_CORE = B_PER_CORE * EDGES_PER_MOL  # 32768

GROUP = 512                      # moving free dim per matmul (fp32 max, 1 PSUM bank)
PSUM_WIDE = 2 * GROUP            # psum tile free size (2 banks; holds 2048 edges)
LN_WIDE = 2 * PSUM_WIDE          # Ln pass width in columns
CHUNK = 4096                     # edges per DMA chunk (2 MiB transfers)
N_CHUNKS = E_PER_CORE // CHUNK   # 8
# One Ln (+accum slot) covers a whole chunk: 2*LN_WIDE = CHUNK edges
# (each column position holds 2 edges via the partition halves).
N_SLOTS = N_CHUNKS               # 8 accumulator slots, slot c == chunk c
SLOTS_PER_MOL = EDGES_PER_MOL // CHUNK  # 4

LOG2 = float(np.log(2.0))

# "native": single ScalarE Softplus LUT pass — NOT supported by this
#   toolchain's act_info.json (no softplus func set) -> walrus lowering fails.
# "explog": two passes, exp then ln(1+t); both funcs live in the
#   natural_log_exp_and_others ACT table set, so no table switching.
SOFTPLUS_MODE = "explog"

_NC_CACHE = {}

# Both halves of softplus = ln(1 + exp(x)) live in this ACT table set. The
# default table-load pass picks the first set containing each function
# (exp -> exp_and_others, ln -> natural_log), which inserts a ~1.3us
# LoadActFuncSet before nearly every activation (~80us/core!). Restricting
# the candidate tables to the combined set keeps one load for the whole
# kernel. Other sets are blanked (not removed) so act_func_set_id indices
# into act_info.json stay valid.
_ACT_SET_BOTH = "natural_log_exp_and_others"


class _EnergyBacc(bacc.Bacc):
    def insert_act_table_loads(self):
        import bass_rust as _bass_rust
        from concourse.hw_specs import get_activation_tables

        has_activation = any(
            isinstance(i, mybir.InstActivation)
            for b in self.main_func.blocks
            for i in b.instructions
        )
        if not has_activation:
            return
        all_tables = get_activation_tables(self.m.arch)
        if _ACT_SET_BOTH in all_tables:
            tables = [
                (name, funcs if name == _ACT_SET_BOTH else set())
                for name, funcs in all_tables.items()
            ]
        else:  # unexpected toolchain: fall back to default behaviour
            tables = list(all_tables.items())
        _bass_rust.insert_act_table_loads(self, tables)


def _chunk_plan(tail_split):
    """Edge counts per chunk. tail_split shortens the serial tail after the
    last DMA by tapering the final chunks. Chunks never straddle a molecule
    boundary and must be multiples of 2*GROUP (1024 edges)."""
    if tail_split == 2:  # finer taper
        return [CHUNK] * (N_CHUNKS - 1) + [CHUNK // 2, CHUNK // 4, CHUNK // 4]
    if tail_split:
        return [CHUNK] * (N_CHUNKS - 1) + [CHUNK // 2, CHUNK // 2]
    return [CHUNK] * N_CHUNKS


def _build_nc(softplus_mode: str, reps: int = 1, loop: int = 0, parts: str = "full",
              xbufs: int = 3, psbufs: int = 3, gbufs: int = 3,
              dma_split: int = 4, tail_split: bool = False,
              dual_dge: bool = False, staggered: bool = False) -> bass.Bass:
    """Build the per-core Bass program. reps>1 repeats the whole kernel body
    unrolled; loop>0 wraps the body in a For_i hardware loop. Both are used
    only for slope-based HW timing; the output is just overwritten."""
    from contextlib import ExitStack

    nc = _EnergyBacc("TRN2", target_bir_lowering=False, debug=False)
    f32 = mybir.dt.float32
    plan = _chunk_plan(tail_split)
    n_slots = len(plan)
    xt = nc.dram_tensor("xt", [F, E_PER_CORE], f32, kind="ExternalInput")
    w1 = nc.dram_tensor("w1", [F, H], f32, kind="ExternalInput")
    b1c = nc.dram_tensor("b1c", [128, 1], f32, kind="ExternalInput")
    acc = nc.dram_tensor("acc", [128, n_slots], f32, kind="ExternalOutput")

    with tile.TileContext(nc) as tc:
        with ExitStack() as ctx:
            consts = ctx.enter_context(tc.tile_pool(name="consts", bufs=1))
            xpool = ctx.enter_context(tc.tile_pool(name="xpool", bufs=xbufs))
            psum = ctx.enter_context(tc.tile_pool(name="psum", bufs=psbufs, space="PSUM"))
            gpool = ctx.enter_context(tc.tile_pool(name="gpool", bufs=gbufs))
            opool = ctx.enter_context(tc.tile_pool(name="opool", bufs=1))

            # Const loads go on the ACT HWDGE ring so they don't sit ahead of
            # the first edge-chunk DMA in the SP ring's FIFO.
            w1_sb = consts.tile([F, H], f32)
            nc.scalar.dma_start(w1_sb[:], w1[:, :])
            b1_sb = consts.tile([128, 1], f32)
            nc.scalar.dma_start(b1_sb[:], b1c[:, :])

            acc_sb = opool.tile([128, n_slots], f32)

            if loop:
                ctx.enter_context(tc.For_i(0, loop, 1, staggered_reset=staggered))

            dma_engines = [nc.sync, nc.scalar] if dual_dge else [nc.sync]
            dma_i = 0
            for _rep in range(reps):
                # Zero-init: makes overwrite-vs-accumulate accum_out semantics
                # equivalent (each slot is written by exactly one instruction).
                nc.vector.memset(acc_sb[:], 0.0)

                e_base = 0
                for c, csize in enumerate(plan):
                    xtile = xpool.tile([F, CHUNK], f32, tag="xtile")
                    nsplit = max(1, min(dma_split, csize // 1024))
                    part = csize // nsplit
                    for s in range(nsplit):
                        eng = dma_engines[dma_i % len(dma_engines)]
                        dma_i += 1
                        eng.dma_start(
                            xtile[:, s * part : (s + 1) * part],
                            xt[:, e_base + s * part : e_base + (s + 1) * part],
                        )
                    e_base += csize
                    if parts == "dma":
                        continue
                    # t accumulates exp() for the whole chunk; one wide Ln
                    # (+free row-sum accum) finishes softplus per chunk.
                    cwide = csize // 2  # columns; 2 edges per column position
                    t = gpool.tile([128, LN_WIDE], f32, tag="t")
                    # each psum tile holds up to 2*PSUM_WIDE edges (2 per
                    # column position, via the partition halves)
                    pos = 0  # edge offset within the chunk
                    while pos < csize:
                        pw = min(PSUM_WIDE, (csize - pos) // 2)
                        ps = psum.tile([128, PSUM_WIDE], f32, tag="ps")
                        # Column-tiled pairs: M=64 matmuls land on disjoint
                        # PSUM partition halves and run concurrently in the
                        # PE array; each [64, 512] output fits one bank.
                        for q in range(pw // GROUP):
                            g0 = pos + 2 * q * GROUP
                            nc.tensor.matmul(
                                ps[0:64, q * GROUP : (q + 1) * GROUP],
                                w1_sb[:], xtile[:, g0 : g0 + GROUP],
                                start=True, stop=True,
                            )
                            nc.tensor.matmul(
                                ps[64:128, q * GROUP : (q + 1) * GROUP],
                                w1_sb[:], xtile[:, g0 + GROUP : g0 + 2 * GROUP],
                                start=True, stop=True,
                            )
                        if parts != "dma+mm":
                            nc.scalar.activation(
                                t[:, pos // 2 : pos // 2 + pw], ps[:, :pw],
                                mybir.ActivationFunctionType.Exp,
                                bias=b1_sb[:], scale=1.0,
                            )
                        pos += 2 * pw
                    if parts == "dma+mm":
                        continue
                    g = gpool.tile([128, LN_WIDE], f32, tag="g")
                    nc.scalar.activation(
                        g[:, :cwide], t[:, :cwide],
                        mybir.ActivationFunctionType.Ln,
                        bias=1.0, scale=1.0,
                        accum_out=acc_sb[:, c : c + 1],
                    )
                # Single final accumulator DMA: per-slot [128, 1] DMAs were
                # tried and HURT (~7us) — 128 four-byte descriptors each,
                # descriptor-dominated, stealing SDMA throughput from the
                # main edge stream.
                nc.sync.dma_start(acc[:, :], acc_sb[:])
    nc.compile()
    return nc


# kernel() uses the tail-split chunk plan: the last 4 MiB chunk becomes two
# 2 MiB chunks, halving the serial matmul->Exp->Ln tail after the final DMA
# (~2-3us off the one-shot execution; steady-state throughput unchanged).
TAIL_SPLIT = True


def _slot_mols(plan):
    """Molecule index owning each accumulator slot (chunks never straddle)."""
    mols, e = [], 0
    for sz in plan:
        mols.append(e // EDGES_PER_MOL)
        e += sz
    return mols


def _get_nc() -> bass.Bass:
    key = (SOFTPLUS_MODE, TAIL_SPLIT)
    if key not in _NC_CACHE:
        _NC_CACHE[key] = _build_nc(SOFTPLUS_MODE, tail_split=TAIL_SPLIT)
    return _NC_CACHE[key]


def _make_in_maps(edge_embedding, W1, b1):
    X = np.ascontiguousarray(edge_embedding, dtype=np.float32).reshape(B, EDGES_PER_MOL, F)
    w1 = np.ascontiguousarray(W1, dtype=np.float32)
    b1c = np.concatenate([np.asarray(b1, np.float32)] * 2).reshape(128, 1)
    b1c = np.ascontiguousarray(b1c)
    in_maps = []
    for c in range(N_CORES):
        xc = X[c * B_PER_CORE : (c + 1) * B_PER_CORE].reshape(E_PER_CORE, F)
        xtc = np.ascontiguousarray(xc.T)  # [F, E] shard, F on partitions
        in_maps.append({"xt": xtc, "w1": w1, "b1c": b1c})
    return in_maps


def _finalize(results, W1, b1, W2, b2):
    W2v = np.asarray(W2, np.float64).reshape(H)
    b2v = float(np.asarray(b2).reshape(()))
    out = np.empty((B, 1), np.float32)
    corr = -EDGES_PER_MOL * LOG2 * float(W2v.sum()) + EDGES_PER_MOL * b2v
    mols = np.array(_slot_mols(_chunk_plan(TAIL_SPLIT)))
    for c in range(N_CORES):
        acc = np.asarray(results[c]["acc"], np.float64)  # [128, n_slots]
        S = acc[0:64, :] + acc[64:128, :]  # per-h, per-slot softplus sums
        for i in range(B_PER_CORE):
            b = c * B_PER_CORE + i
            Sg = S[:, mols == i].sum(axis=1)
            out[b, 0] = np.float32(Sg @ W2v + corr)
    return out


def kernel_with_results(edge_embedding, W1, b1, W2, b2, trace=False, **run_kwargs):
    nc = _get_nc()
    in_maps = _make_in_maps(edge_embedding, W1, b1)
    core_ids = list(range(N_CORES))
    try:
        br = run_bass_kernel_spmd(nc, in_maps, core_ids, trace=trace, **run_kwargs)
    except ModuleNotFoundError:
        # Slim axon clients lack the NTFF profile hook (antenv.axon_hooks);
        # retry without tracing rather than failing the whole kernel.
        import os
        os.environ["BASS_NEVER_TRACE"] = "1"
        br = run_bass_kernel_spmd(nc, in_maps, core_ids, trace=False, **run_kwargs)
    out = _finalize(br.results, W1, b1, W2, b2)
    return out, br


def kernel(edge_embedding, W1, b1, W2, b2):
    out, _ = kernel_with_results(edge_embedding, W1, b1, W2, b2)
    return out
